# revision 25
# baseline (speedup 1.0000x reference)
"""Bass/Tile kernel for causal self-attention, head-sharded across cores.

Per-core layout (core c owns heads 2c, 2c+1):
  inputs (per core):
    xT    [C, B*T]        bf16   x transposed (feature-major), same on all cores
    wqkv  [128, KC, F]    bf16   W_qkv column-slice, [p, kchunk, f]; f = [q_h0|q_h1|k_h0|k_h1|v_h0|v_h1] * 64
    wproj [128, C]        bf16   W_proj row-slice (rows = this core's 128 head dims)
    bqkv  [128, FC]       f32    b_qkv slice, partition-major per f-chunk
    pbias [128, B, T/128] f32    key-padding bias (0 or -1e30), partition-major per key chunk
    mask1 [128, 128]      bf16   0/1 local causal triangle (p <= j), shared by all diagonal offsets
  output:
    outT  [C, B*T]        bf16   partial projection output (pre-bias), feature-major

Structure (v2):
  - emission per (batch, 512-token block): QKV projection block immediately
    followed by that block's attention; the Tile scheduler interleaves QKV /
    proj matmuls into attention's ACT-bound stretches.
  - diagonal key-chunks are column-narrowed: for diagonal offset o only query
    columns >= o*128 are computed in S, exp, mask, PV and denominator ops
    (causal masking makes the rest exactly zero contribution).
  - exp is one ACT op per chunk over both heads ([128, 2, n] AP); key-padding
    bias rides the ACT bias port (per-partition = per-key, same for both heads).
  - causal mask applied by a single [128, 2, 128] DVE multiply on the diagonal
    128-column strip only.
  - partial outputs stored as bf16, one batched DMA per 512-token block.
"""

import concourse.bass as bass
import concourse.mybir as mybir
import concourse.tile as tile
from concourse import bacc

F32 = mybir.dt.float32
BF16 = mybir.dt.bfloat16
AF = mybir.ActivationFunctionType


def build_nc(B=4, T=2048, C=1024, HPC=2, D=64, TB=512, num_devices=8,
             scale=None, pad_bias=True):
    if scale is None:
        scale = D ** -0.5
    NT = B * T                 # total tokens
    NB = NT // TB              # 512-token blocks (global)
    BPB = T // TB              # blocks per batch (4)
    CPB = TB // 128            # 128-chunks per block (4)
    NCH = T // 128             # key chunks per batch (16)
    KC = C // 128              # contraction chunks for qkv matmul (8)
    F = HPC * 3 * D            # per-core qkv features (384)
    FC = F // 128              # f-chunks (3)
    assert HPC == 2 and HPC * D == 128 and F % 128 == 0 and TB % 128 == 0

    nc = bacc.Bacc("TRN2", target_bir_lowering=False, debug=False,
                   num_devices=num_devices)

    xT = nc.dram_tensor("xT", [C, NT], BF16, kind="ExternalInput")
    wqkv = nc.dram_tensor("wqkv", [128, KC, F], BF16, kind="ExternalInput")
    wproj = nc.dram_tensor("wproj", [128, C], BF16, kind="ExternalInput")
    bqkv = nc.dram_tensor("bqkv", [128, FC], F32, kind="ExternalInput")
    pbias = nc.dram_tensor("pbias", [128, B, NCH], F32, kind="ExternalInput")
    mask1 = nc.dram_tensor("mask1", [128, 128], BF16, kind="ExternalInput")
    outT = nc.dram_tensor("outT", [C, NT], BF16, kind="ExternalOutput")

    with tile.TileContext(nc) as tc:
        with (
            tc.tile_pool(name="const", bufs=1) as const,
            tc.tile_pool(name="persist", bufs=1) as persist,
            tc.tile_pool(name="xp", bufs=2) as xp,
            tc.tile_pool(name="pp", bufs=10) as pp,
            tc.tile_pool(name="rp", bufs=3) as rp,
            tc.tile_pool(name="sp", bufs=6) as sp,
            tc.tile_pool(name="op", bufs=3) as op,
            tc.tile_pool(name="psmm", bufs=2, space="PSUM") as psmm,
            tc.tile_pool(name="pso", bufs=1, space="PSUM") as pso,
            tc.tile_pool(name="psd", bufs=1, space="PSUM") as psd,
            tc.tile_pool(name="pss", bufs=2, space="PSUM") as pss,
        ):
            # ---- constants (tiles created here; DMAs emitted in load order) ----
            w_sb = const.tile([128, KC, F], BF16, tag="w", name="w_sb")
            bq_sb = const.tile([128, FC], F32, tag="bq", name="bq_sb")
            pb_sb = const.tile([128, B, NCH], F32, tag="pb", name="pb_sb")
            mk_sb = const.tile([128, HPC, 128], BF16, tag="mk", name="mk_sb")
            ones_sb = const.tile([128, 64], BF16, tag="ones", name="ones_sb")
            nc.vector.memset(ones_sb[:], 1.0)
            wp_sb = const.tile([128, C], BF16, tag="wp", name="wp_sb")
            # preload ACT exp table during the DMA head phase
            warm_sb = const.tile([128, 1], F32, tag="warm", name="warm_sb")
            nc.scalar.activation(out=warm_sb[:], in_=ones_sb[:, 0:1],
                                 func=AF.Exp)

            # ---- persistent per-block tiles ----
            qT = [persist.tile([128, TB], BF16, tag=f"qT{i}", name=f"qT{i}")
                  for i in range(NB)]
            kT = [persist.tile([128, TB], BF16, tag=f"kT{i}", name=f"kT{i}")
                  for i in range(NB)]
            V = [persist.tile([128, CPB, 128], BF16, tag=f"V{i}", name=f"V{i}")
                 for i in range(NB)]

            def load_x(b, w_interleave=False):
                # one [128, KC, T] tile per batch; two half-loads per kc on
                # different DGE paths. For b==0: first-block quarter on
                # gpsimd ahead of everything so matmul #0's deps land first.
                xt = xp.tile([128, KC, T], BF16, tag="xt", name=f"x{b}")
                if w_interleave:
                    # first block's deps first, split across both queues
                    for kc in range(KC):
                        r0 = kc * 128
                        nc.gpsimd.dma_start(out=w_sb[:, kc, :],
                                            in_=wqkv[:, kc, :])
                        nc.sync.dma_start(
                            out=xt[:, kc, 0:TB],
                            in_=xT[r0:r0 + 128, b * T:b * T + TB])
                    nc.gpsimd.dma_start(out=bq_sb[:], in_=bqkv[:])
                    nc.gpsimd.dma_start(out=pb_sb[:], in_=pbias[:])
                    for h in range(HPC):
                        nc.gpsimd.dma_start(out=mk_sb[:, h, :], in_=mask1[:])
                    for kc in range(KC):
                        r0 = kc * 128
                        nc.gpsimd.dma_start(
                            out=xt[:, kc, TB:T // 2],
                            in_=xT[r0:r0 + 128, b * T + TB:b * T + T // 2])
                        nc.sync.dma_start(
                            out=xt[:, kc, T // 2:3 * T // 4],
                            in_=xT[r0:r0 + 128,
                                   b * T + T // 2:b * T + 3 * T // 4])
                    for kc in range(KC):
                        r0 = kc * 128
                        nc.sync.dma_start(
                            out=xt[:, kc, 3 * T // 4:T],
                            in_=xT[r0:r0 + 128, b * T + 3 * T // 4:(b + 1) * T])
                    nc.sync.dma_start(out=wp_sb[:], in_=wproj[:])
                    return xt
                for kc in range(KC):
                    r0 = kc * 128
                    nc.gpsimd.dma_start(
                        out=xt[:, kc, 0:T // 2],
                        in_=xT[r0:r0 + 128, b * T:b * T + T // 2])
                    nc.sync.dma_start(
                        out=xt[:, kc, T // 2:T],
                        in_=xT[r0:r0 + 128, b * T + T // 2:(b + 1) * T])
                return xt

            def qkv_units(xt, b, qb):
                """Emit-callables for one block's QKV projection, split into
                half-chains (~4 matmuls each) usable as PE gap fillers inside
                the ACT-bound attention pair loop."""
                tb = b * BPB + qb
                state = {}

                def chain(fc):
                    def emit():
                        ps = psmm.tile([128, TB], F32, tag="ps", name="ps")
                        for kc in range(KC):
                            nc.tensor.matmul(
                                ps[:],
                                lhsT=w_sb[:, kc, fc * 128:(fc + 1) * 128],
                                rhs=xt[:, kc, qb * TB:(qb + 1) * TB],
                                start=(kc == 0), stop=(kc == KC - 1))
                        if fc == 0:
                            dest = qT[tb]
                        elif fc == 1:
                            dest = kT[tb]
                        else:
                            dest = persist.tile([128, TB], BF16,
                                                tag=f"vs{tb % 2}", name="vs")
                        nc.any.tensor_scalar_add(
                            out=dest[:], in0=ps[:], scalar1=bq_sb[:, fc:fc + 1])
                        if fc == 2:
                            nc.sync.dma_start_transpose(out=V[tb][:],
                                                        in_=dest[:])
                    return emit

                return [chain(fc) for fc in range(FC)]

            def proj_units(b, qb, at):
                """Emit-callables for one block's output projection (pairs of
                matmuls + psum evacuation), plus the batched store."""
                gb = b * BPB + qb
                ot = op.tile([128, KC, TB], BF16, tag="ot", name="ot")

                def pair(f0):
                    def emit():
                        for fc in (f0, f0 + 1):
                            ps = psmm.tile([128, TB], F32, tag="ps", name="ps")
                            nc.tensor.matmul(
                                ps[:], lhsT=wp_sb[:, fc * 128:(fc + 1) * 128],
                                rhs=at[:], start=True, stop=True)
                            nc.any.tensor_copy(ot[:, fc, :], ps[:])
                        eng = nc.gpsimd if (f0 // 2) % 2 == 0 else nc.sync
                        eng.dma_start(
                            out=outT[f0 * 128:(f0 + 2) * 128,
                                     gb * TB:(gb + 1) * TB].rearrange(
                                "(f p) t -> p f t", f=2),
                            in_=ot[:, f0:f0 + 2, :])
                    return emit

                return [pair(f0) for f0 in range(0, KC, 2)]

            def attn_chunks(b, qb, psO, psD, fq, pop_unit):
                gb = b * BPB + qb
                nchunks = (qb + 1) * CPB
                assert nchunks % 2 == 0
                npairs = nchunks // 2
                pend = [None]   # pending off-diag pair-sum (quad den fusion)
                for p in range(npairs):
                    c0 = 2 * p
                    pts, col0s = [], []
                    for ci in (c0, c0 + 1):
                        cb = ci // CPB      # kT block within batch
                        cl = ci % CPB       # 128-chunk within that block
                        o = ci - qb * CPB   # diagonal offset (>=0: diagonal)
                        col0 = max(0, o) * 128
                        ktile = kT[b * BPB + cb]
                        psS = pss.tile([128, HPC, TB], F32, tag="pss",
                                       name="psS")
                        for h in range(HPC):
                            nc.tensor.matmul(
                                psS[:, h, col0:TB],
                                lhsT=ktile[h * 64:(h + 1) * 64,
                                           cl * 128:(cl + 1) * 128],
                                rhs=qT[gb][h * 64:(h + 1) * 64, col0:TB],
                                start=True, stop=True,
                                tile_position=(h * 64, 0))
                        pt = pp.tile([128, HPC, TB], BF16, tag="pt", name="pt")
                        if pad_bias:
                            nc.scalar.activation(
                                out=pt[:, :, col0:TB], in_=psS[:, :, col0:TB],
                                func=AF.Exp,
                                bias=pb_sb[:, b, ci:ci + 1], scale=scale)
                        else:
                            nc.scalar.activation(
                                out=pt[:, :, col0:TB], in_=psS[:, :, col0:TB],
                                func=AF.Exp, scale=scale)
                        if o >= 0:  # diagonal: mask the 128-wide strip
                            nc.any.tensor_mul(
                                pt[:, :, col0:col0 + 128],
                                pt[:, :, col0:col0 + 128], mk_sb[:])
                        pts.append(pt)
                        col0s.append(col0)
                    # filler between S emission and PV emission: PE chews
                    # independent qkv/proj matmuls while ACT runs the exps
                    if fq:
                        pop_unit()
                    for j, ci in enumerate((c0, c0 + 1)):
                        vtile = V[b * BPB + ci // CPB]
                        col0 = col0s[j]
                        for h in range(HPC):
                            nc.tensor.matmul(
                                psO[h * 64:(h + 1) * 64, col0:TB],
                                lhsT=vtile[:, ci % CPB, h * 64:(h + 1) * 64],
                                rhs=pts[j][:, h, col0:TB],
                                start=(ci == 0), stop=(ci == nchunks - 1),
                                tile_position=(0, h * 64))
                    if fq and len(fq) > npairs - p:
                        pop_unit()
                    if c0 + 1 < qb * CPB:
                        # off-diagonal pair: defer denominator via pair-sum;
                        # every second pair completes a 4-chunk quad -> one
                        # denominator matmul pair instead of four
                        s = sp.tile([128, HPC, TB], BF16, tag="s", name="s")
                        nc.any.tensor_add(s[:], pts[0][:], pts[1][:])
                        if pend[0] is None:
                            pend[0] = (s, c0 == 0)
                        else:
                            s0, is_first = pend[0]
                            pend[0] = None
                            s2 = sp.tile([128, HPC, TB], BF16, tag="s",
                                         name="s2")
                            nc.any.tensor_add(s2[:], s0[:], s[:])
                            for h in range(HPC):
                                nc.tensor.matmul(
                                    psD[h * 64:(h + 1) * 64, :],
                                    lhsT=ones_sb[:], rhs=s2[:, h, :],
                                    start=is_first, stop=False,
                                    tile_position=(0, h * 64))
                    else:
                        for j, ci in enumerate((c0, c0 + 1)):
                            col0 = col0s[j]
                            for h in range(HPC):
                                nc.tensor.matmul(
                                    psD[h * 64:(h + 1) * 64, col0:TB],
                                    lhsT=ones_sb[:],
                                    rhs=pts[j][:, h, col0:TB],
                                    start=(ci == 0), stop=(ci == nchunks - 1),
                                    tile_position=(0, h * 64))
                assert pend[0] is None

            from collections import deque
            blocks = [(b, qb) for b in range(B) for qb in range(BPB)]
            xts = {0: load_x(0, w_interleave=True)}
            for u in qkv_units(xts[0], 0, 0):
                u()
            # filler queue holds (block_idx, fn); qkv units are queued two
            # blocks ahead so batch-boundary blocks never starve the PE
            fq = deque()
            pending_qkv = [0] * (NB + 2)

            def queue_qkv(j):
                if j < NB:
                    jb, jqb = blocks[j]
                    for u in qkv_units(xts[jb], jb, jqb):
                        fq.append((j, u))
                        pending_qkv[j] += 1

            def pop_unit():
                j, u = fq.popleft()
                if j >= 0:
                    pending_qkv[j] -= 1
                u()

            queue_qkv(1)
            for i, (b, qb) in enumerate(blocks):
                if qb == 1 and b + 1 < B:
                    xts[b + 1] = load_x(b + 1)
                queue_qkv(i + 2)
                psO = pso.tile([128, TB], F32, tag="psO", name="psO")
                psD = psd.tile([128, TB], F32, tag="psD", name="psD")
                attn_chunks(b, qb, psO, psD, fq, pop_unit)
                # normalize first (frees psO/psD early), then seam work
                rt = rp.tile([128, TB], F32, tag="rt", name="rt")
                nc.vector.reciprocal_approx_fast(out=rt[:], in_=psD[:])
                at = rp.tile([128, TB], BF16, tag="at", name="at")
                nc.vector.tensor_mul(at[:], psO[:], rt[:])
                # seam: ensure next block's qkv is fully emitted
                while pending_qkv[i + 1] > 0:
                    pop_unit()
                for u in proj_units(b, qb, at):
                    fq.append((-1, u))
            while fq:
                pop_unit()

    nc.compile()
    return nc


def prep_core_inputs(x, key_padding_mask, W_qkv, b_qkv, W_proj,
                     n_cores=8, TB=512):
    """Host-side sharding: build the per-core input maps."""
    import numpy as np
    import ml_dtypes

    B, T, C = x.shape
    D = 64
    H = C // D
    HPC = H // n_cores
    BT = B * T

    xT = np.ascontiguousarray(
        x.reshape(BT, C).T).astype(ml_dtypes.bfloat16)          # [C, BT]

    pb = np.where(key_padding_mask, np.float32(-1e30),
                  np.float32(0.0)).astype(np.float32)           # [B, T]
    pb = np.ascontiguousarray(pb.reshape(B, T // 128, 128).transpose(2, 0, 1))

    p = np.arange(128)[:, None]
    j = np.arange(128)[None, :]
    mk = (p <= j).astype(ml_dtypes.bfloat16)                    # [128, 128]

    KC = C // 128
    in_maps = []
    for c in range(n_cores):
        hs = [HPC * c + i for i in range(HPC)]
        cols = np.concatenate([
            np.concatenate([which * H * D + h * D + np.arange(D) for h in hs])
            for which in range(3)])                             # [F]
        Wc = W_qkv[:, cols]                                     # [C, F]
        F = Wc.shape[1]
        wq = np.ascontiguousarray(
            Wc.reshape(KC, 128, F).transpose(1, 0, 2)).astype(ml_dtypes.bfloat16)
        bq = np.ascontiguousarray(
            b_qkv[cols].reshape(F // 128, 128).T).astype(np.float32)
        rows = np.concatenate([h * D + np.arange(D) for h in hs])
        wp = np.ascontiguousarray(W_proj[rows, :]).astype(ml_dtypes.bfloat16)
        in_maps.append({
            "xT": xT, "wqkv": wq.reshape(128, KC, F), "wproj": wp,
            "bqkv": bq, "pbias": pb, "mask1": mk,
        })
    return in_maps


def combine_outputs(results, B, T, C, b_proj):
    import numpy as np
    acc = results[0]["outT"].astype(np.float32)
    for r in results[1:]:
        acc = acc + r["outT"].astype(np.float32)
    out = acc.T.reshape(B, T, C) + b_proj.astype(np.float32)
    return out.astype(np.float32)


# ---------------------------------------------------------------------------
# Self-contained entry point for the grading harness.
# kernel(**inputs) takes the FULL unsharded inputs and returns the FULL output.
# Sharding: tensor-parallel over heads (2 heads per core, 8 cores); each core
# computes its QKV column-slice, attention for its heads, and a partial output
# projection; partials are summed on the host.
# ---------------------------------------------------------------------------
import numpy as np

_NC_CACHE = {}


def _get_nc():
    if "nc" not in _NC_CACHE:
        _NC_CACHE["nc"] = build_nc(B=4, T=2048, C=1024, num_devices=8)
    return _NC_CACHE["nc"]


def kernel(x, key_padding_mask, W_qkv, b_qkv, W_proj, b_proj):
    from concourse.bass_utils import run_bass_kernel_spmd

    x = np.asarray(x, dtype=np.float32)
    key_padding_mask = np.asarray(key_padding_mask).astype(bool)
    W_qkv = np.asarray(W_qkv, dtype=np.float32)
    b_qkv = np.asarray(b_qkv, dtype=np.float32)
    W_proj = np.asarray(W_proj, dtype=np.float32)
    b_proj = np.asarray(b_proj, dtype=np.float32)

    B, T, C = x.shape
    nc = _get_nc()
    in_maps = prep_core_inputs(x, key_padding_mask, W_qkv, b_qkv, W_proj,
                               n_cores=8)
    res = run_bass_kernel_spmd(nc, in_maps, list(range(8)))
    return combine_outputs(res.results, B, T, C, b_proj)
